# revision 1
# baseline (speedup 1.0000x reference)
"""Distributed AttentionLayer kernel for one TRN2 chip (8 NeuronCores).

Reference computation (note the unusual softmax over the QUERY axis):
    Q = Xq @ Wq.T + bq                      [B, L, 128]
    K = Xk @ Wk.T + bk
    V = Xv @ Wv.T + bv
    S = softmax(Q @ K.T / sqrt(128), axis=q)    (normalized over queries)
    H = S @ V                               [B, L, 128]

Sharding: 8 cores = 4 batches x 2 key-chunks.  Core i handles batch
b = i // 2 and keys [2048*h, 2048*h + 2048), h = i % 2, with the FULL
query range.  Because the softmax normalizer sums over q (fully local)
and H = sum_k E[q,k]/colsum[k] * V[k,v] splits cleanly over k, each core
computes an exact partial H with ZERO collectives; the host adds the two
k-chunk partials per batch.

On-core dataflow (everything transposed so contractions land on the
partition axis; host pre-transposes/bf16-casts the input shards):
    QT[o,q]  = sum_d WqT[d,o].T @ XqT[d,q]   (+bq per-partition, fused in evac)
    KT[o,k]  likewise
    V[k,v]   = sum_d XvT[d,k].T @ WvT[d,v]   (+bv via a rank-1 ones x bv matmul)
    ST[k,q]  = KT[:,ktile].T @ QT            (contraction over o=128)
    ET       = exp(ST/sqrt(128)) (bf16), accum_out gives colsum over q free axis
    V'[k,v]  = V * (1/colsum[k])             (per-partition scalar)
    HT[v,q]  = sum_kt V'[ktile].T @ ET[ktile]
Output per core: HT [128, 4096] f32; host: H[b] = (HT_even + HT_odd).T
"""

import math

import numpy as np
import ml_dtypes

B, L, DM, DH = 4, 4096, 1024, 128
NCORES = 8
KCH = L // 2            # 2048 keys per core
QCS = 512               # query chunk (one PSUM bank of f32)
NQC = L // QCS          # 8
NKT = KCH // 128        # 16 key tiles per core
NDT = DM // 128         # 8 d_model tiles
NKC = KCH // QCS        # 4 key 512-chunks for the K projection
SCALE = 1.0 / math.sqrt(DH)

_CACHE = {}


def _build():
    import concourse.tile as tile
    from concourse import bacc, mybir

    f32 = mybir.dt.float32
    bf16 = mybir.dt.bfloat16
    AX = mybir.AxisListType
    ALU = mybir.AluOpType
    ACT = mybir.ActivationFunctionType

    nc = bacc.Bacc("TRN2", target_bir_lowering=False, debug=False,
                   num_devices=NCORES)

    # Host-side layouts (see kernel() below):
    #   xq_t: [qc, p, dt, c]  with d = dt*128+p, q = qc*512+c  (8KB/partition per DMA)
    #   xk_t/xv_t: [dt, p, k] with d = dt*128+p
    #   w*_t: [dt, p, o]      (W.T tiled over d)
    xq_d = nc.dram_tensor("xq_t", [NQC, 128, NDT, QCS], bf16, kind="ExternalInput")
    xk_d = nc.dram_tensor("xk_t", [NDT, 128, KCH], bf16, kind="ExternalInput")
    xv_d = nc.dram_tensor("xv_t", [NDT, 128, KCH], bf16, kind="ExternalInput")
    wq_d = nc.dram_tensor("wq_t", [NDT, 128, DH], bf16, kind="ExternalInput")
    wk_d = nc.dram_tensor("wk_t", [NDT, 128, DH], bf16, kind="ExternalInput")
    wv_d = nc.dram_tensor("wv_t", [NDT, 128, DH], bf16, kind="ExternalInput")
    bq_d = nc.dram_tensor("bq", [DH, 1], f32, kind="ExternalInput")
    bk_d = nc.dram_tensor("bk", [DH, 1], f32, kind="ExternalInput")
    bv_d = nc.dram_tensor("bv", [1, DH], bf16, kind="ExternalInput")
    out_d = nc.dram_tensor("out", [DH, L], f32, kind="ExternalOutput")

    with tile.TileContext(nc) as tc:
        with tc.tile_pool(name="const", bufs=1) as cpool, \
             tc.tile_pool(name="persist", bufs=1) as ppool, \
             tc.tile_pool(name="psmm", bufs=4, space="PSUM") as psmm, \
             tc.tile_pool(name="psv", bufs=2, space="PSUM") as psv, \
             tc.tile_pool(name="psh", bufs=2, space="PSUM") as psh:

            # ---------- constants ----------
            wq_sb = cpool.tile([128, NDT, DH], bf16, name="wq_sb", tag="wq")
            wk_sb = cpool.tile([128, NDT, DH], bf16, name="wk_sb", tag="wk")
            wv_sb = cpool.tile([128, NDT, DH], bf16, name="wv_sb", tag="wv")
            bq_sb = cpool.tile([128, 1], f32, name="bq_sb", tag="bq")
            bk_sb = cpool.tile([128, 1], f32, name="bk_sb", tag="bk")
            bv_sb = cpool.tile([1, DH], bf16, name="bv_sb", tag="bv")
            ones_sb = cpool.tile([1, DH], bf16, name="ones_sb", tag="ones")

            nc.sync.dma_start(out=wq_sb[:], in_=wq_d[:].rearrange("t p c -> p t c"))
            nc.sync.dma_start(out=wk_sb[:], in_=wk_d[:].rearrange("t p c -> p t c"))
            nc.sync.dma_start(out=wv_sb[:], in_=wv_d[:].rearrange("t p c -> p t c"))
            nc.sync.dma_start(out=bq_sb[:], in_=bq_d[:])
            nc.sync.dma_start(out=bk_sb[:], in_=bk_d[:])
            nc.sync.dma_start(out=bv_sb[:], in_=bv_d[:])
            nc.vector.memset(ones_sb[:], 1.0)

            # ---------- persistent activations ----------
            qt_sb = ppool.tile([128, L], bf16, name="qt_sb", tag="qt")      # Q^T [o, q]
            kt_sb = ppool.tile([128, KCH], bf16, name="kt_sb", tag="kt")    # K^T [o, k]
            v_sb = ppool.tile([128, NKT, DH], bf16, name="v_sb", tag="v")   # V  [k, kt, v]
            vs_sb = ppool.tile([128, NKT, DH], bf16, name="vs_sb", tag="vs")
            cs_parts = ppool.tile([128, NKT, NQC], f32, name="cs_parts", tag="csp")
            cs_sum = ppool.tile([128, NKT], f32, name="cs_sum", tag="css")
            cs_rec = ppool.tile([128, NKT], f32, name="cs_rec", tag="csr")

            # ---------- load X shards + projections ----------
            with tc.tile_pool(name="xin", bufs=1) as xpool:
                xk_sb = xpool.tile([128, NDT, KCH], bf16, name="xk_sb", tag="xk")
                xv_sb = xpool.tile([128, NDT, KCH], bf16, name="xv_sb", tag="xv")
                xq_sbs = []
                for qc in range(NQC):
                    t = xpool.tile([128, NDT, QCS], bf16, name=f"xq_sb{qc}",
                                   tag=f"xq{qc}")
                    xq_sbs.append(t)

                for dt in range(NDT):
                    nc.sync.dma_start(out=xk_sb[:, dt, :], in_=xk_d[dt])
                for qc in range(NQC):
                    nc.sync.dma_start(out=xq_sbs[qc][:], in_=xq_d[qc])
                for dt in range(NDT):
                    nc.sync.dma_start(out=xv_sb[:, dt, :], in_=xv_d[dt])

                # K^T projection: KT[o, k] += WqT_tile.T @ XkT
                for kc in range(NKC):
                    kt_ps = psmm.tile([128, QCS], f32, name=f"kt_ps{kc}", tag="mm512")
                    for dt in range(NDT):
                        nc.tensor.matmul(
                            out=kt_ps[:],
                            lhsT=wk_sb[:, dt, :],
                            rhs=xk_sb[:, dt, kc * QCS:(kc + 1) * QCS],
                            start=(dt == 0), stop=(dt == NDT - 1))
                    nc.vector.tensor_scalar_add(
                        out=kt_sb[:, kc * QCS:(kc + 1) * QCS], in0=kt_ps[:],
                        scalar1=bk_sb[:, 0:1])

                # Q^T projection
                for qc in range(NQC):
                    qt_ps = psmm.tile([128, QCS], f32, name=f"qt_ps{qc}", tag="mm512")
                    for dt in range(NDT):
                        nc.tensor.matmul(
                            out=qt_ps[:],
                            lhsT=wq_sb[:, dt, :],
                            rhs=xq_sbs[qc][:, dt, :],
                            start=(dt == 0), stop=(dt == NDT - 1))
                    nc.vector.tensor_scalar_add(
                        out=qt_sb[:, qc * QCS:(qc + 1) * QCS], in0=qt_ps[:],
                        scalar1=bq_sb[:, 0:1])

                # V projection: V[k, v] = bv (rank-1) + sum_dt XvT_tile.T @ WvT_tile
                for kt in range(NKT):
                    v_ps = psv.tile([128, DH], f32, name=f"v_ps{kt}", tag="vps")
                    nc.tensor.matmul(out=v_ps[:], lhsT=ones_sb[:], rhs=bv_sb[:],
                                     start=True, stop=False)
                    for dt in range(NDT):
                        nc.tensor.matmul(
                            out=v_ps[:],
                            lhsT=xv_sb[:, dt, kt * 128:(kt + 1) * 128],
                            rhs=wv_sb[:, dt, :],
                            start=False, stop=(dt == NDT - 1))
                    nc.vector.tensor_copy(out=v_sb[:, kt, :], in_=v_ps[:])

            # ---------- scores + exp (+ fused colsum over q) ----------
            with tc.tile_pool(name="etp", bufs=NKT) as epool:
                et_ts = []
                for kt in range(NKT):
                    t = epool.tile([128, L], bf16, name=f"et_sb{kt}", tag="et",
                                   bufs=NKT)
                    et_ts.append(t)

                for kt in range(NKT):
                    for qc in range(NQC):
                        st_ps = psmm.tile([128, QCS], f32,
                                          name=f"st_ps_{kt}_{qc}", tag="mm512")
                        nc.tensor.matmul(
                            out=st_ps[:],
                            lhsT=kt_sb[:, kt * 128:(kt + 1) * 128],
                            rhs=qt_sb[:, qc * QCS:(qc + 1) * QCS],
                            start=True, stop=True)
                        nc.scalar.activation(
                            out=et_ts[kt][:, qc * QCS:(qc + 1) * QCS],
                            in_=st_ps[:], func=ACT.Exp, scale=SCALE,
                            accum_out=cs_parts[:, kt, qc:qc + 1])

                # colsum -> reciprocal -> fold into V
                for kt in range(NKT):
                    nc.vector.tensor_reduce(
                        out=cs_sum[:, kt:kt + 1], in_=cs_parts[:, kt, :],
                        axis=AX.X, op=ALU.add)
                nc.vector.reciprocal(out=cs_rec[:], in_=cs_sum[:])
                for kt in range(NKT):
                    nc.vector.tensor_scalar_mul(
                        out=vs_sb[:, kt, :], in0=v_sb[:, kt, :],
                        scalar1=cs_rec[:, kt:kt + 1])

                # ---------- H^T = sum_kt V'.T @ ET ----------
                for qc in range(NQC):
                    ht_ps = psh.tile([128, QCS], f32, name=f"ht_ps{qc}", tag="ht")
                    for kt in range(NKT):
                        nc.tensor.matmul(
                            out=ht_ps[:],
                            lhsT=vs_sb[:, kt, :],
                            rhs=et_ts[kt][:, qc * QCS:(qc + 1) * QCS],
                            start=(kt == 0), stop=(kt == NKT - 1))
                    ht_sb = ppool.tile([128, QCS], f32, name=f"ht_sb{qc}",
                                       tag="htsb", bufs=2)
                    nc.vector.tensor_copy(out=ht_sb[:], in_=ht_ps[:])
                    nc.sync.dma_start(out=out_d[:, qc * QCS:(qc + 1) * QCS],
                                      in_=ht_sb[:])

    nc.compile()
    return nc


def _get_nc():
    if "nc" not in _CACHE:
        _CACHE["nc"] = _build()
    return _CACHE["nc"]


def _make_in_maps(inp_q, inp_k, inp_v, Wq, bq, Wk, bk, Wv, bv):
    bf = ml_dtypes.bfloat16
    f32 = np.float32

    def wt(W):  # [128, 1024] -> W.T tiled [dt, p, o], bf16
        return np.ascontiguousarray(W.T.reshape(NDT, 128, DH)).astype(bf)

    wq_np, wk_np, wv_np = wt(Wq), wt(Wk), wt(Wv)
    bq_np = np.ascontiguousarray(bq.reshape(DH, 1)).astype(f32)
    bk_np = np.ascontiguousarray(bk.reshape(DH, 1)).astype(f32)
    bv_np = np.ascontiguousarray(bv.reshape(1, DH)).astype(bf)

    in_maps = []
    for b in range(B):
        # Xq[b].T -> [qc, p, dt, c]
        xq_np = (inp_q[b].T.reshape(NDT, 128, NQC, QCS)
                 .transpose(2, 1, 0, 3).astype(bf))
        xq_np = np.ascontiguousarray(xq_np)
        for h in range(2):
            sl = slice(h * KCH, (h + 1) * KCH)
            xk_np = np.ascontiguousarray(
                inp_k[b, sl].T.reshape(NDT, 128, KCH)).astype(bf)
            xv_np = np.ascontiguousarray(
                inp_v[b, sl].T.reshape(NDT, 128, KCH)).astype(bf)
            in_maps.append({
                "xq_t": xq_np, "xk_t": xk_np, "xv_t": xv_np,
                "wq_t": wq_np, "wk_t": wk_np, "wv_t": wv_np,
                "bq": bq_np, "bk": bk_np, "bv": bv_np,
            })
    return in_maps


def kernel(inp_q, inp_k, inp_v, Wq, bq, Wk, bk, Wv, bv, _trace=False):
    from concourse.bass_utils import run_bass_kernel_spmd

    inp_q = np.asarray(inp_q, np.float32)
    inp_k = np.asarray(inp_k, np.float32)
    inp_v = np.asarray(inp_v, np.float32)
    Wq, bq = np.asarray(Wq, np.float32), np.asarray(bq, np.float32)
    Wk, bk = np.asarray(Wk, np.float32), np.asarray(bk, np.float32)
    Wv, bv = np.asarray(Wv, np.float32), np.asarray(bv, np.float32)

    nc = _get_nc()
    in_maps = _make_in_maps(inp_q, inp_k, inp_v, Wq, bq, Wk, bk, Wv, bv)
    res = run_bass_kernel_spmd(nc, in_maps, core_ids=list(range(NCORES)),
                               trace=_trace)
    if _trace:
        _CACHE["last_result"] = res

    H = np.empty((B, L, DH), np.float32)
    for b in range(B):
        H[b] = (res.results[2 * b]["out"] + res.results[2 * b + 1]["out"]).T
    return H
